# revision 19
# baseline (speedup 1.0000x reference)
"""LocalAttention1d Trainium2 kernel.

Layout strategy (B=16 sharded over 8 cores, 2 batches/core):
  - p_t chain in ~fp32 precision: h = tanh(c@W_p.T) via fp16x2 split matmuls
    (c = c1+c2 fp16 pair, W likewise; 3 cross terms give ~1e-7 rel accuracy),
    logit = <tanh(h), V_p> via fused DVE multiply-reduce in fp32.
  - windowed gather: p_int -> int16 block-start indices -> SWDGE dma_gather
    of 7 contiguous q^T rows (one 7KB descriptor per t) from DRAM fp16.
  - scores: fused DVE multiply-reduce (fp16) against u = c@W_a.
  - softmax*gauss -> 7 diagonal fp16 matmuls accumulate the weighted sum in
    PSUM (t-partition layout).
  - software pipeline over 6 variable-size stages (tile groups 2/3/3|3/3/2)
    with staggered emission so PE/DVE/GPSIMD queues overlap across stages.
"""

import sys

sys.path.insert(0, "/opt/trn_rl_repo")

import numpy as np

import bass_rust
import concourse.bass as bass
import concourse.tile as tile
from concourse import bacc, mybir
from concourse.bass_utils import run_bass_kernel_spmd

B, T, S, QS, CS, PS, D = 16, 1024, 4096, 512, 512, 512, 3
NCORE = 8
BPC = B // NCORE  # batches per core
NJ = 2 * D + 1  # 7 window positions
NT = T // 128  # 8 t-tiles per batch
STAGES = [(0, [0, 1]), (0, [2, 3]), (0, [4, 5, 6, 7]),
          (1, [0, 1, 2, 3]), (1, [4, 5]), (1, [6, 7])]
NSTG = len(STAGES)
NHM = 4  # max tiles per stage

dt = mybir.dt
AF = mybir.ActivationFunctionType
ALU = mybir.AluOpType

LAST_EXEC_NS = None
LAST_RES = None
_CACHE = {}


def _build_nc():
    nc = bacc.Bacc("TRN2", target_bir_lowering=False, debug=False, num_devices=NCORE)

    qT16 = nc.dram_tensor("qT16", [BPC, S, QS], dt.float16, kind="ExternalInput").ap()
    cT1 = nc.dram_tensor("cT1", [BPC, CS, T], dt.float16, kind="ExternalInput").ap()
    cT2 = nc.dram_tensor("cT2", [BPC, CS, T], dt.float16, kind="ExternalInput").ap()
    wp1 = nc.dram_tensor("wp1", [CS, PS], dt.float16, kind="ExternalInput").ap()
    wp2 = nc.dram_tensor("wp2", [CS, PS], dt.float16, kind="ExternalInput").ap()
    wa1 = nc.dram_tensor("wa1", [CS, QS], dt.float16, kind="ExternalInput").ap()
    vpr = nc.dram_tensor("vpr", [128, PS], dt.float32, kind="ExternalInput").ap()
    offs = nc.dram_tensor("offs", [128, NT * NJ], dt.float32, kind="ExternalInput").ap()
    modm = nc.dram_tensor("modm", [128, 128], dt.float32, kind="ExternalInput").ap()
    blk16 = nc.dram_tensor("blk16", [128, 8], dt.float32, kind="ExternalInput").ap()
    id128h = nc.dram_tensor("id128h", [128, 128], dt.float16, kind="ExternalInput").ap()
    out = nc.dram_tensor("out", [BPC, T, QS], dt.float16, kind="ExternalOutput").ap()

    with tile.TileContext(nc) as tc:
        import contextlib

        ctx = contextlib.ExitStack()
        with ctx:
            cpool = ctx.enter_context(tc.tile_pool(name="consts", bufs=1))
            ctp = ctx.enter_context(tc.tile_pool(name="ct", bufs=16))
            gp = ctx.enter_context(tc.tile_pool(name="gath", bufs=12))
            up = ctx.enter_context(tc.tile_pool(name="u16", bufs=9))
            sp = ctx.enter_context(tc.tile_pool(name="small", bufs=3))
            dp = ctx.enter_context(tc.tile_pool(name="dall", bufs=2))
            gtp = ctx.enter_context(tc.tile_pool(name="tanh", bufs=2))
            jp = ctx.enter_context(tc.tile_pool(name="junk", bufs=2))
            op = ctx.enter_context(tc.tile_pool(name="outp", bufs=2))
            mmp = ctx.enter_context(tc.tile_pool(name="mm", bufs=2, space="PSUM"))
            wsp = ctx.enter_context(tc.tile_pool(name="ws", bufs=2, space="PSUM"))
            tpp = ctx.enter_context(tc.tile_pool(name="tp", bufs=2, space="PSUM"))

            def _floor_into(dst, src, sfx, shape):
                """Exact floor(src) for src >= 0 written into dst; src/dst are
                equally-shaped APs, scratch allocated at `shape` and sliced."""
                sl = (slice(None),) + tuple(slice(0, n) for n in src.shape[1:])
                i32 = sp.tile(shape, dt.int32, tag="fli" + sfx)
                iv = i32[sl]
                nc.vector.tensor_copy(iv, src)
                cand = sp.tile(shape, dt.float32, tag="flc" + sfx)
                cv = cand[sl]
                nc.vector.tensor_copy(cv, iv)
                corr = sp.tile(shape, dt.float32, tag="flx" + sfx)
                rv = corr[sl]
                nc.vector.scalar_tensor_tensor(rv, cv, 1.0, src, ALU.bypass, ALU.is_gt)
                nc.vector.tensor_tensor(dst, cv, rv, ALU.subtract)

            # ---- constants + c loads on sync queue, dependency order ----
            wp1t = cpool.tile([128, 4, PS], dt.float16)
            wp2t = cpool.tile([128, 4, PS], dt.float16)
            for k in range(4):
                nc.sync.dma_start(wp1t[:, k, :], wp1[k * 128 : (k + 1) * 128, :])
                nc.sync.dma_start(wp2t[:, k, :], wp2[k * 128 : (k + 1) * 128, :])

            all_ct1s, all_ct2s = [[], []], [[], []]
            for b in range(BPC):
                for k in range(4):
                    c1t = ctp.tile([128, T], dt.float16, tag="ct1")
                    nc.scalar.dma_start(c1t[:], cT1[b, k * 128 : (k + 1) * 128, :])
                    all_ct1s[b].append(c1t)
                for k in range(4):
                    c2t = ctp.tile([128, T], dt.float16, tag="ct2")
                    nc.scalar.dma_start(c2t[:], cT2[b, k * 128 : (k + 1) * 128, :])
                    all_ct2s[b].append(c2t)
            vprt = cpool.tile([128, PS], dt.float32)
            nc.sync.dma_start(vprt[:], vpr[:])
            modmt = cpool.tile([128, 128], dt.float32)
            nc.sync.dma_start(modmt[:], modm[:])
            blk16t = cpool.tile([128, 8], dt.float32)
            nc.sync.dma_start(blk16t[:], blk16[:])
            wa1t = cpool.tile([128, 4, QS], dt.float16)
            nc.sync.dma_start(wa1t[:], wa1[:].rearrange("(k p) n -> p k n", p=128))
            offst = cpool.tile([128, NT * NJ], dt.float32)
            nc.sync.dma_start(offst[:], offs[:])
            id128ht = cpool.tile([128, 128], dt.float16)
            nc.sync.dma_start(id128ht[:], id128h[:])

            def chunk(t, k):
                return t[:, k, :]

            qwins = []
            for b in range(BPC):
                qw = qT16[b].copy()
                qw.ap = bass_rust.VecI64Pair([[QS, S - NJ + 1], [1, NJ * QS]])
                qwins.append(qw)

            st = [dict() for _ in range(NSTG)]

            def phase_A(s):
                """h matmuls + tanh + logit + perm/idx chain for one stage."""
                b, ms = STAGES[s]
                nh = len(ms)
                ct1s, ct2s = all_ct1s[b], all_ct2s[b]
                lg = sp.tile([128, NHM], dt.float32, tag="lg")
                st[s]["lg"] = lg
                for i, m in enumerate(ms):
                    hps = mmp.tile([128, PS], dt.float32, tag="hps", space="PSUM")
                    nmm = 0
                    terms = [
                        (ct1s[k][:, m * 128 : (m + 1) * 128], chunk(w, k))
                        for k in range(4)
                        for w in (wp1t, wp2t)
                    ] + [
                        (ct2s[k][:, m * 128 : (m + 1) * 128], chunk(wp1t, k))
                        for k in range(4)
                    ]
                    for lhs, rhs in terms:
                        nc.tensor.matmul(hps[:], lhs, rhs, start=(nmm == 0), stop=(nmm == 11))
                        nmm += 1
                    g = gtp.tile([128, PS], dt.float32, tag="g")
                    nc.scalar.activation(g[:], hps[:], AF.Tanh)
                    junkf = jp.tile([128, PS], dt.float16, tag="junkf")
                    nc.vector.scalar_tensor_tensor(
                        junkf[:], g[:], 1.0, vprt[:], ALU.bypass, ALU.mult,
                        accum_out=lg[:, i : i + 1],
                    )
                # t-layout p_t chain once: sigmoid -> x4096 -> exact floor
                sig8 = sp.tile([128, NHM], dt.float32, tag="sig8")
                nc.scalar.activation(sig8[:, :nh], lg[:, :nh], AF.Sigmoid)
                i32 = sp.tile([128, NHM], dt.int32, tag="fli8")
                nc.vector.tensor_scalar_mul(i32[:, :nh], sig8[:, :nh], 4096.0)
                cand = sp.tile([128, NHM], dt.float32, tag="flc8")
                nc.vector.tensor_copy(cand[:, :nh], i32[:, :nh])
                corr = sp.tile([128, NHM], dt.float32, tag="flx8")
                nc.vector.scalar_tensor_tensor(
                    corr[:, :nh], cand[:, :nh], 1.0 / 4096.0, sig8[:, :nh],
                    ALU.mult, ALU.is_gt,
                )
                pi8 = sp.tile([128, NHM], dt.float32, tag="pi8")
                nc.vector.tensor_tensor(pi8[:, :nh], cand[:, :nh], corr[:, :nh], ALU.subtract)
                pt8 = sp.tile([128, NHM], dt.float32, tag="pt8")
                nc.vector.tensor_scalar_mul(pt8[:, :nh], sig8[:, :nh], 4096.0)
                st[s]["pi8"] = pi8
                st[s]["pt8"] = pt8
                # wrapped-16 permute of the floored p_int via one mask-matmul:
                # pw[p, (i,w)] = sum_p' 1[p' % 16 == p % 16] (pi8[p',i] blk[p',w])
                prh = sp.tile([128, NHM, 8], dt.float32, tag="prh")
                nc.vector.tensor_tensor(
                    prh[:, :nh, :],
                    pi8[:, :nh, None].broadcast_to([128, nh, 8]),
                    blk16t[:, None, :].broadcast_to([128, nh, 8]),
                    ALU.mult,
                )
                pps = tpp.tile([128, NHM, 8], dt.float32, tag="pps", space="PSUM")
                nc.tensor.matmul(
                    pps[:, :nh, :], modmt[:], prh[:, :nh, :], start=True, stop=True
                )
                idxs = sp.tile([128, NHM, 8], dt.int16, tag="idxs")
                tmpp = sp.tile([128, NHM, 8], dt.float32, tag="tmpp")
                nc.vector.tensor_scalar(
                    tmpp[:, :nh, :], pps[:, :nh, :], 3.0, 4092.0, ALU.max, ALU.min
                )
                nc.vector.tensor_scalar(
                    idxs[:, :nh, :], tmpp[:, :nh, :], -3.0, None, ALU.add
                )
                st[s]["idxs"] = idxs

            def phase_B(s):
                """gathers (gpsimd queue) + t-layout gauss/mask prep."""
                b, ms = STAGES[s]
                nh = len(ms)
                idxs = st[s]["idxs"]
                gts = []
                for i, m in enumerate(ms):
                    gt = gp.tile([128, 1, NJ * QS], dt.float16, tag="gt")
                    nc.gpsimd.dma_gather(
                        gt[:], qwins[b], idxs[:, i, :], 128, 128, NJ * QS,
                        elem_step=QS, single_packet=False,
                    )
                    gts.append(gt[:, 0, :].rearrange("p (j q) -> p j q", j=NJ))
                st[s]["gts"] = gts

                pi8, pt8 = st[s]["pi8"], st[s]["pt8"]

                pos_all = sp.tile([128, NHM, NJ], dt.float32, tag="pos_all")
                pos3 = pos_all[:, :nh, :]
                nc.vector.scalar_tensor_tensor(
                    pos3, pi8[:, :nh, None].broadcast_to([128, nh, NJ]), 1.0,
                    offst[:].rearrange("p (m j) -> p m j", j=NJ)[:, :nh, :],
                    ALU.bypass, ALU.add,
                )
                dtile = sp.tile([128, NHM, NJ], dt.float32, tag="dtile")
                nc.vector.scalar_tensor_tensor(
                    dtile[:, :nh, :],
                    pt8[:, :nh, None].broadcast_to([128, nh, NJ]), 1.0,
                    pos3, ALU.bypass, ALU.subtract,
                )
                # gauss = exp(-(2/9) d^2); square on DVE keeps ACT table set small
                g1 = sp.tile([128, NHM, NJ], dt.float32, tag="g1")
                nc.vector.tensor_tensor(
                    g1[:, :nh, :], dtile[:, :nh, :], dtile[:, :nh, :], ALU.mult
                )
                gauss = sp.tile([128, NHM, NJ], dt.float32, tag="gauss")
                nc.scalar.activation(gauss[:, :nh, :], g1[:, :nh, :], AF.Exp, scale=-2.0 / 9.0)
                m1 = sp.tile([128, NHM, NJ], dt.float32, tag="m1")
                nc.vector.tensor_scalar(
                    m1[:, :nh, :], pos_all[:, :nh, :], 0.0, -1e30, ALU.is_lt, ALU.mult
                )
                maskb = sp.tile([128, NHM, NJ], dt.float32, tag="maskb")
                nc.vector.tensor_scalar(
                    maskb[:, :nh, :], pos_all[:, :nh, :], 4095.0, -1e30, ALU.is_gt, ALU.mult
                )
                nc.vector.tensor_add(maskb[:, :nh, :], maskb[:, :nh, :], m1[:, :nh, :])
                st[s]["gauss"] = gauss
                st[s]["maskb"] = maskb

            def phase_C(s):
                """u = c1 @ W_a for the stage (PE, overlaps gather DMA)."""
                b, ms = STAGES[s]
                ct1s = all_ct1s[b]
                u16s = []
                for m in ms:
                    ups = mmp.tile([128, QS], dt.float32, tag="ups", space="PSUM")
                    for k in range(4):
                        nc.tensor.matmul(
                            ups[:], ct1s[k][:, m * 128 : (m + 1) * 128], chunk(wa1t, k),
                            start=(k == 0), stop=(k == 3),
                        )
                    u16 = up.tile([128, QS], dt.float16, tag="u16")
                    nc.scalar.activation(u16[:], ups[:], AF.Copy)
                    u16s.append(u16)
                st[s]["u16s"] = u16s

            def phase_D(s):
                """scores: fused multiply-reduce per (tile, j)."""
                nh = len(STAGES[s][1])
                gts, u16s = st[s]["gts"], st[s]["u16s"]
                a_h = sp.tile([128, NHM, NJ], dt.float32, tag="a_h")
                for i in range(nh):
                    for j in range(NJ):
                        junk16 = jp.tile([128, QS], dt.float16, tag="junk16")
                        nc.vector.scalar_tensor_tensor(
                            junk16[:], gts[i][:, j, :], 1.0, u16s[i][:],
                            ALU.bypass, ALU.mult,
                            accum_out=a_h[:, i, j : j + 1],
                        )
                st[s]["a_h"] = a_h

            def phase_E(s):
                """masked softmax * gauss -> fp16 diag weights."""
                nh = len(STAGES[s][1])
                a_h, gauss, maskb = st[s]["a_h"], st[s]["gauss"], st[s]["maskb"]
                a3 = a_h[:, :nh, :]
                nc.vector.tensor_add(a3, a3, maskb[:, :nh, :])
                rmax = sp.tile([128, NHM], dt.float32, tag="rmax")
                nc.vector.tensor_reduce(rmax[:, :nh, None], a3, mybir.AxisListType.X, ALU.max)
                asub = sp.tile([128, NHM, NJ], dt.float32, tag="asub")
                nc.vector.scalar_tensor_tensor(
                    asub[:, :nh, :],
                    rmax[:, :nh, None].broadcast_to([128, nh, NJ]), 1.0,
                    a3, ALU.bypass, ALU.subtract,
                )
                e_h = sp.tile([128, NHM, NJ], dt.float32, tag="e_h")
                nc.scalar.activation(e_h[:, :nh, :], asub[:, :nh, :], AF.Exp, scale=-1.0)
                rsum = sp.tile([128, NHM], dt.float32, tag="rsum")
                nc.vector.tensor_reduce(
                    rsum[:, :nh, None], e_h[:, :nh, :], mybir.AxisListType.X, ALU.add
                )
                rinv = sp.tile([128, NHM], dt.float32, tag="rinv")
                nc.vector.reciprocal(rinv[:, :nh], rsum[:, :nh])
                wt = sp.tile([128, NHM, NJ], dt.float32, tag="wt")
                nc.vector.scalar_tensor_tensor(
                    wt[:, :nh, :],
                    rinv[:, :nh, None].broadcast_to([128, nh, NJ]), 1.0,
                    e_h[:, :nh, :], ALU.bypass, ALU.mult,
                )
                nc.vector.tensor_mul(wt[:, :nh, :], wt[:, :nh, :], gauss[:, :nh, :])
                wt16 = sp.tile([128, NHM, NJ], dt.float16, tag="wt16")
                nc.vector.tensor_copy(wt16[:, :nh, :], wt[:, :nh, :])
                dall = dp.tile([128, NHM, NJ, 128], dt.float16, tag="dall")
                nc.vector.tensor_tensor(
                    dall[:, :nh, :, :],
                    id128ht[:, None, None, :].broadcast_to([128, nh, NJ, 128]),
                    wt16[:, :nh, :, None].broadcast_to([128, nh, NJ, 128]),
                    ALU.mult,
                )
                st[s]["dall"] = dall

            def phase_F(s):
                """weighted sum via diagonal fp16 matmuls + store."""
                b, ms = STAGES[s]
                gts, dall = st[s]["gts"], st[s]["dall"]
                for i, m in enumerate(ms):
                    wps = wsp.tile([128, QS], dt.float32, tag="wps", space="PSUM")
                    for j in range(NJ):
                        nc.tensor.matmul(
                            wps[:], dall[:, i, j, :], gts[i][:, j, :],
                            start=(j == 0), stop=(j == NJ - 1),
                        )
                    outt = op.tile([128, QS], dt.float16, tag="outt")
                    nc.scalar.activation(outt[:], wps[:], AF.Copy)
                    nc.sync.dma_start(out[b, m * 128 : (m + 1) * 128, :], outt[:])

            # ---- staggered emission: overlap stages across engine queues ----
            sched = []
            for s in range(NSTG):
                sched += [(phase_A, s), (phase_B, s), (phase_C, s)]
                if s >= 1:
                    sched += [(phase_D, s - 1), (phase_E, s - 1)]
                if s >= 2:
                    sched += [(phase_F, s - 2)]
            sched += [(phase_D, NSTG - 1), (phase_E, NSTG - 1)]
            sched += [(phase_F, NSTG - 2), (phase_F, NSTG - 1)]
            for fn, s in sched:
                fn(s)

    nc.compile()
    return nc


def _host_prep(q, c_t, W_a, W_p, V_p):
    q = np.asarray(q, dtype=np.float32)
    c_t = np.asarray(c_t, dtype=np.float32)
    W_a = np.asarray(W_a, dtype=np.float32)
    W_p = np.asarray(W_p, dtype=np.float32)
    V_p = np.asarray(V_p, dtype=np.float32)

    qT16 = np.ascontiguousarray(q.transpose(0, 2, 1)).astype(np.float16)
    cT = np.ascontiguousarray(c_t.transpose(0, 2, 1))
    cT1 = cT.astype(np.float16)
    cT2 = (cT - cT1.astype(np.float32)).astype(np.float16)
    wpT = np.ascontiguousarray(W_p.T)
    wp1 = wpT.astype(np.float16)
    wp2 = (wpT - wp1.astype(np.float32)).astype(np.float16)
    wa1 = W_a.astype(np.float16)
    vpr = np.ascontiguousarray(np.tile(V_p.reshape(1, PS), (128, 1)), dtype=np.float32)
    offs = np.tile(np.arange(-3, 4, dtype=np.float32).reshape(1, 1, NJ), (128, NT, 1))
    offs = np.ascontiguousarray(offs.reshape(128, NT * NJ))
    modm = np.zeros((128, 128), dtype=np.float32)
    for pp in range(128):
        for p in range(128):
            if pp % 16 == p % 16:
                modm[pp, p] = 1.0
    blk16 = np.zeros((128, 8), dtype=np.float32)
    for pp in range(128):
        blk16[pp, pp // 16] = 1.0
    id128h = np.eye(128).astype(np.float16)

    consts = dict(wp1=wp1, wp2=wp2, wa1=wa1, vpr=vpr, offs=offs, modm=modm,
                  blk16=blk16, id128h=id128h)
    in_maps = []
    for k in range(NCORE):
        sl = slice(k * BPC, (k + 1) * BPC)
        m = dict(consts)
        m["qT16"] = np.ascontiguousarray(qT16[sl])
        m["cT1"] = np.ascontiguousarray(cT1[sl])
        m["cT2"] = np.ascontiguousarray(cT2[sl])
        in_maps.append(m)
    return in_maps


def kernel(q, c_t, W_a, W_p, V_p):
    global LAST_EXEC_NS, LAST_RES
    if "nc" not in _CACHE:
        _CACHE["nc"] = _build_nc()
    nc = _CACHE["nc"]
    in_maps = _host_prep(q, c_t, W_a, W_p, V_p)
    res = run_bass_kernel_spmd(nc, in_maps, core_ids=list(range(NCORE)))
    LAST_RES = res
    LAST_EXEC_NS = res.exec_time_ns
    outs = [res.results[k]["out"] for k in range(NCORE)]
    return np.concatenate(outs, axis=0).astype(np.float32)


# revision 21
# speedup vs baseline: 1.0239x; 1.0239x over previous
"""LocalAttention1d Trainium2 kernel.

Layout strategy (B=16 sharded over 8 cores, 2 batches/core):
  - p_t chain in ~fp32 precision: h = tanh(c@W_p.T) via fp16x2 split matmuls
    (c = c1+c2 fp16 pair, W likewise; 3 cross terms give ~1e-7 rel accuracy),
    logit = <tanh(h), V_p> via fused DVE multiply-reduce in fp32.
  - windowed gather: p_int -> int16 block-start indices -> SWDGE dma_gather
    of 7 contiguous q^T rows (one 7KB descriptor per t) from DRAM fp16.
  - scores: fused DVE multiply-reduce (fp16) against u = c@W_a.
  - softmax*gauss -> 7 diagonal fp16 matmuls accumulate the weighted sum in
    PSUM (t-partition layout).
  - software pipeline over 6 variable-size stages (tile groups 2/3/3|3/3/2)
    with staggered emission so PE/DVE/GPSIMD queues overlap across stages.
"""

import sys

sys.path.insert(0, "/opt/trn_rl_repo")

import numpy as np

import bass_rust
import concourse.bass as bass
import concourse.tile as tile
from concourse import bacc, mybir
from concourse.bass_utils import run_bass_kernel_spmd

B, T, S, QS, CS, PS, D = 16, 1024, 4096, 512, 512, 512, 3
NCORE = 8
BPC = B // NCORE  # batches per core
NJ = 2 * D + 1  # 7 window positions
NT = T // 128  # 8 t-tiles per batch
STAGES = [(0, [0, 1]), (0, [2, 3]), (0, [4, 5, 6, 7]),
          (1, [0, 1, 2, 3]), (1, [4, 5]), (1, [6, 7])]
NSTG = len(STAGES)
NHM = 4  # max tiles per stage

dt = mybir.dt
AF = mybir.ActivationFunctionType
ALU = mybir.AluOpType

LAST_EXEC_NS = None
LAST_RES = None
_CACHE = {}


def _build_nc():
    nc = bacc.Bacc("TRN2", target_bir_lowering=False, debug=False, num_devices=NCORE)

    qT16 = nc.dram_tensor("qT16", [BPC, S, QS], dt.float16, kind="ExternalInput").ap()
    cT1 = nc.dram_tensor("cT1", [BPC, CS, T], dt.float16, kind="ExternalInput").ap()
    cT2 = nc.dram_tensor("cT2", [BPC, CS, T], dt.float16, kind="ExternalInput").ap()
    wp1 = nc.dram_tensor("wp1", [CS, PS], dt.float16, kind="ExternalInput").ap()
    wp2 = nc.dram_tensor("wp2", [CS, PS], dt.float16, kind="ExternalInput").ap()
    wa1 = nc.dram_tensor("wa1", [CS, QS], dt.float16, kind="ExternalInput").ap()
    vpr = nc.dram_tensor("vpr", [128, PS], dt.float32, kind="ExternalInput").ap()
    offs = nc.dram_tensor("offs", [128, NT * NJ], dt.float32, kind="ExternalInput").ap()
    modm = nc.dram_tensor("modm", [128, 128], dt.float32, kind="ExternalInput").ap()
    blk16 = nc.dram_tensor("blk16", [128, 8], dt.float32, kind="ExternalInput").ap()
    id128h = nc.dram_tensor("id128h", [128, 128], dt.float16, kind="ExternalInput").ap()
    out = nc.dram_tensor("out", [BPC, T, QS], dt.float16, kind="ExternalOutput").ap()

    with tile.TileContext(nc) as tc:
        import contextlib

        ctx = contextlib.ExitStack()
        with ctx:
            cpool = ctx.enter_context(tc.tile_pool(name="consts", bufs=1))
            ctp = ctx.enter_context(tc.tile_pool(name="ct", bufs=16))
            gp = ctx.enter_context(tc.tile_pool(name="gath", bufs=11))
            up = ctx.enter_context(tc.tile_pool(name="u16", bufs=9))
            sp = ctx.enter_context(tc.tile_pool(name="small", bufs=3))
            dp = ctx.enter_context(tc.tile_pool(name="dall", bufs=2))
            gtp = ctx.enter_context(tc.tile_pool(name="tanh", bufs=2))
            jp = ctx.enter_context(tc.tile_pool(name="junk", bufs=3))
            op = ctx.enter_context(tc.tile_pool(name="outp", bufs=2))
            mmp = ctx.enter_context(tc.tile_pool(name="mm", bufs=2, space="PSUM"))
            wsp = ctx.enter_context(tc.tile_pool(name="ws", bufs=2, space="PSUM"))
            tpp = ctx.enter_context(tc.tile_pool(name="tp", bufs=2, space="PSUM"))

            def _floor_into(dst, src, sfx, shape):
                """Exact floor(src) for src >= 0 written into dst; src/dst are
                equally-shaped APs, scratch allocated at `shape` and sliced."""
                sl = (slice(None),) + tuple(slice(0, n) for n in src.shape[1:])
                i32 = sp.tile(shape, dt.int32, tag="fli" + sfx)
                iv = i32[sl]
                nc.vector.tensor_copy(iv, src)
                cand = sp.tile(shape, dt.float32, tag="flc" + sfx)
                cv = cand[sl]
                nc.vector.tensor_copy(cv, iv)
                corr = sp.tile(shape, dt.float32, tag="flx" + sfx)
                rv = corr[sl]
                nc.vector.scalar_tensor_tensor(rv, cv, 1.0, src, ALU.bypass, ALU.is_gt)
                nc.vector.tensor_tensor(dst, cv, rv, ALU.subtract)

            # ---- constants + c loads on sync queue, dependency order ----
            wp1t = cpool.tile([128, 4, PS], dt.float16)
            wp2t = cpool.tile([128, 4, PS], dt.float16)
            for k in range(4):
                nc.sync.dma_start(wp1t[:, k, :], wp1[k * 128 : (k + 1) * 128, :])
                nc.sync.dma_start(wp2t[:, k, :], wp2[k * 128 : (k + 1) * 128, :])

            all_ct1s, all_ct2s = [[], []], [[], []]
            for k in range(4):
                c1t = ctp.tile([128, T], dt.float16, tag="ct1")
                nc.sync.dma_start(c1t[:], cT1[0, k * 128 : (k + 1) * 128, :])
                all_ct1s[0].append(c1t)
            for k in range(4):
                c2t = ctp.tile([128, T], dt.float16, tag="ct2")
                nc.sync.dma_start(c2t[:], cT2[0, k * 128 : (k + 1) * 128, :])
                all_ct2s[0].append(c2t)
            vprt = cpool.tile([128, PS], dt.float32)
            nc.sync.dma_start(vprt[:], vpr[:])
            modmt = cpool.tile([128, 128], dt.float32)
            nc.sync.dma_start(modmt[:], modm[:])
            blk16t = cpool.tile([128, 8], dt.float32)
            nc.sync.dma_start(blk16t[:], blk16[:])
            wa1t = cpool.tile([128, 4, QS], dt.float16)
            nc.sync.dma_start(wa1t[:], wa1[:].rearrange("(k p) n -> p k n", p=128))
            offst = cpool.tile([128, NT * NJ], dt.float32)
            nc.sync.dma_start(offst[:], offs[:])
            id128ht = cpool.tile([128, 128], dt.float16)
            nc.sync.dma_start(id128ht[:], id128h[:])
            for k in range(4):
                c1t = ctp.tile([128, T], dt.float16, tag="ct1")
                nc.sync.dma_start(c1t[:], cT1[1, k * 128 : (k + 1) * 128, :])
                all_ct1s[1].append(c1t)
            for k in range(4):
                c2t = ctp.tile([128, T], dt.float16, tag="ct2")
                nc.sync.dma_start(c2t[:], cT2[1, k * 128 : (k + 1) * 128, :])
                all_ct2s[1].append(c2t)

            def chunk(t, k):
                return t[:, k, :]

            qwins = []
            for b in range(BPC):
                qw = qT16[b].copy()
                qw.ap = bass_rust.VecI64Pair([[QS, S - NJ + 1], [1, NJ * QS]])
                qwins.append(qw)

            st = [dict() for _ in range(NSTG)]

            def phase_A(s):
                """h matmuls + tanh + logit + perm/idx chain for one stage."""
                b, ms = STAGES[s]
                nh = len(ms)
                ct1s, ct2s = all_ct1s[b], all_ct2s[b]
                lg = sp.tile([128, NHM], dt.float32, tag="lg")
                st[s]["lg"] = lg
                for i, m in enumerate(ms):
                    hps = mmp.tile([128, PS], dt.float32, tag="hps", space="PSUM")
                    nmm = 0
                    terms = [
                        (ct1s[k][:, m * 128 : (m + 1) * 128], chunk(w, k))
                        for k in range(4)
                        for w in (wp1t, wp2t)
                    ] + [
                        (ct2s[k][:, m * 128 : (m + 1) * 128], chunk(wp1t, k))
                        for k in range(4)
                    ]
                    for lhs, rhs in terms:
                        nc.tensor.matmul(hps[:], lhs, rhs, start=(nmm == 0), stop=(nmm == 11))
                        nmm += 1
                    g = gtp.tile([128, PS], dt.float32, tag="g")
                    nc.scalar.activation(g[:], hps[:], AF.Tanh)
                    junkf = jp.tile([128, PS], dt.float16, tag="junkf")
                    nc.vector.scalar_tensor_tensor(
                        junkf[:], g[:], 1.0, vprt[:], ALU.bypass, ALU.mult,
                        accum_out=lg[:, i : i + 1],
                    )
                # t-layout p_t chain once: sigmoid -> x4096 -> exact floor
                sig8 = sp.tile([128, NHM], dt.float32, tag="sig8")
                nc.scalar.activation(sig8[:, :nh], lg[:, :nh], AF.Sigmoid)
                i32 = sp.tile([128, NHM], dt.int32, tag="fli8")
                nc.vector.tensor_scalar_mul(i32[:, :nh], sig8[:, :nh], 4096.0)
                cand = sp.tile([128, NHM], dt.float32, tag="flc8")
                nc.vector.tensor_copy(cand[:, :nh], i32[:, :nh])
                corr = sp.tile([128, NHM], dt.float32, tag="flx8")
                nc.vector.scalar_tensor_tensor(
                    corr[:, :nh], cand[:, :nh], 1.0 / 4096.0, sig8[:, :nh],
                    ALU.mult, ALU.is_gt,
                )
                pi8 = sp.tile([128, NHM], dt.float32, tag="pi8")
                nc.vector.tensor_tensor(pi8[:, :nh], cand[:, :nh], corr[:, :nh], ALU.subtract)
                pt8 = sp.tile([128, NHM], dt.float32, tag="pt8")
                nc.vector.tensor_scalar_mul(pt8[:, :nh], sig8[:, :nh], 4096.0)
                st[s]["pi8"] = pi8
                st[s]["pt8"] = pt8
                # wrapped-16 permute of the floored p_int via one mask-matmul:
                # pw[p, (i,w)] = sum_p' 1[p' % 16 == p % 16] (pi8[p',i] blk[p',w])
                prh = sp.tile([128, NHM, 8], dt.float32, tag="prh")
                nc.vector.tensor_tensor(
                    prh[:, :nh, :],
                    pi8[:, :nh, None].broadcast_to([128, nh, 8]),
                    blk16t[:, None, :].broadcast_to([128, nh, 8]),
                    ALU.mult,
                )
                pps = tpp.tile([128, NHM, 8], dt.float32, tag="pps", space="PSUM")
                nc.tensor.matmul(
                    pps[:, :nh, :], modmt[:], prh[:, :nh, :], start=True, stop=True
                )
                idxs = sp.tile([128, NHM, 8], dt.int16, tag="idxs")
                tmpp = sp.tile([128, NHM, 8], dt.float32, tag="tmpp")
                nc.vector.tensor_scalar(
                    tmpp[:, :nh, :], pps[:, :nh, :], 3.0, 4092.0, ALU.max, ALU.min
                )
                nc.vector.tensor_scalar(
                    idxs[:, :nh, :], tmpp[:, :nh, :], -3.0, None, ALU.add
                )
                st[s]["idxs"] = idxs

            def phase_B(s):
                """gathers (gpsimd queue) + t-layout gauss/mask prep."""
                b, ms = STAGES[s]
                nh = len(ms)
                idxs = st[s]["idxs"]
                gts = []
                for i, m in enumerate(ms):
                    gt = gp.tile([128, 1, NJ * QS], dt.float16, tag="gt")
                    nc.gpsimd.dma_gather(
                        gt[:], qwins[b], idxs[:, i, :], 128, 128, NJ * QS,
                        elem_step=QS, single_packet=False,
                    )
                    gts.append(gt[:, 0, :].rearrange("p (j q) -> p j q", j=NJ))
                st[s]["gts"] = gts

                pi8, pt8 = st[s]["pi8"], st[s]["pt8"]

                pos_all = sp.tile([128, NHM, NJ], dt.float32, tag="pos_all")
                pos3 = pos_all[:, :nh, :]
                nc.vector.scalar_tensor_tensor(
                    pos3, pi8[:, :nh, None].broadcast_to([128, nh, NJ]), 1.0,
                    offst[:].rearrange("p (m j) -> p m j", j=NJ)[:, :nh, :],
                    ALU.bypass, ALU.add,
                )
                dtile = sp.tile([128, NHM, NJ], dt.float32, tag="dtile")
                nc.vector.scalar_tensor_tensor(
                    dtile[:, :nh, :],
                    pt8[:, :nh, None].broadcast_to([128, nh, NJ]), 1.0,
                    pos3, ALU.bypass, ALU.subtract,
                )
                # gauss = exp(-(2/9) d^2); square on DVE keeps ACT table set small
                g1 = sp.tile([128, NHM, NJ], dt.float32, tag="g1")
                nc.vector.tensor_tensor(
                    g1[:, :nh, :], dtile[:, :nh, :], dtile[:, :nh, :], ALU.mult
                )
                gauss = sp.tile([128, NHM, NJ], dt.float32, tag="gauss")
                nc.scalar.activation(gauss[:, :nh, :], g1[:, :nh, :], AF.Exp, scale=-2.0 / 9.0)
                m1 = sp.tile([128, NHM, NJ], dt.float32, tag="m1")
                nc.vector.tensor_scalar(
                    m1[:, :nh, :], pos_all[:, :nh, :], 0.0, -1e30, ALU.is_lt, ALU.mult
                )
                maskb = sp.tile([128, NHM, NJ], dt.float32, tag="maskb")
                nc.vector.tensor_scalar(
                    maskb[:, :nh, :], pos_all[:, :nh, :], 4095.0, -1e30, ALU.is_gt, ALU.mult
                )
                nc.vector.tensor_add(maskb[:, :nh, :], maskb[:, :nh, :], m1[:, :nh, :])
                st[s]["gauss"] = gauss
                st[s]["maskb"] = maskb

            def phase_C(s):
                """u = c1 @ W_a for the stage (PE, overlaps gather DMA)."""
                b, ms = STAGES[s]
                ct1s = all_ct1s[b]
                u16s = []
                for m in ms:
                    ups = mmp.tile([128, QS], dt.float32, tag="ups", space="PSUM")
                    for k in range(4):
                        nc.tensor.matmul(
                            ups[:], ct1s[k][:, m * 128 : (m + 1) * 128], chunk(wa1t, k),
                            start=(k == 0), stop=(k == 3),
                        )
                    u16 = up.tile([128, QS], dt.float16, tag="u16")
                    nc.scalar.activation(u16[:], ups[:], AF.Copy)
                    u16s.append(u16)
                st[s]["u16s"] = u16s

            def phase_D(s):
                """scores: fused multiply-reduce per (tile, j)."""
                nh = len(STAGES[s][1])
                gts, u16s = st[s]["gts"], st[s]["u16s"]
                a_h = sp.tile([128, NHM, NJ], dt.float32, tag="a_h")
                for i in range(nh):
                    for j in range(NJ):
                        col = a_h[:, i, j : j + 1]
                        if (i * NJ + j) % 2 == 0:
                            junk16 = jp.tile([128, QS], dt.float16, tag="junk16")
                            nc.vector.scalar_tensor_tensor(
                                junk16[:], gts[i][:, j, :], 1.0, u16s[i][:],
                                ALU.bypass, ALU.mult, accum_out=col,
                            )
                        else:
                            prod = jp.tile([128, QS], dt.float16, tag="prod")
                            nc.vector.tensor_tensor(
                                prod[:], gts[i][:, j, :], u16s[i][:], ALU.mult
                            )
                            junka = jp.tile([128, QS], dt.float16, tag="junka")
                            nc.scalar.activation(
                                junka[:], prod[:], AF.Copy, accum_out=col
                            )
                st[s]["a_h"] = a_h

            def phase_E(s):
                """masked softmax * gauss -> fp16 diag weights."""
                nh = len(STAGES[s][1])
                a_h, gauss, maskb = st[s]["a_h"], st[s]["gauss"], st[s]["maskb"]
                a3 = a_h[:, :nh, :]
                nc.vector.tensor_add(a3, a3, maskb[:, :nh, :])
                rmax = sp.tile([128, NHM], dt.float32, tag="rmax")
                nc.vector.tensor_reduce(rmax[:, :nh, None], a3, mybir.AxisListType.X, ALU.max)
                asub = sp.tile([128, NHM, NJ], dt.float32, tag="asub")
                nc.vector.scalar_tensor_tensor(
                    asub[:, :nh, :],
                    rmax[:, :nh, None].broadcast_to([128, nh, NJ]), 1.0,
                    a3, ALU.bypass, ALU.subtract,
                )
                e_h = sp.tile([128, NHM, NJ], dt.float32, tag="e_h")
                nc.scalar.activation(e_h[:, :nh, :], asub[:, :nh, :], AF.Exp, scale=-1.0)
                rsum = sp.tile([128, NHM], dt.float32, tag="rsum")
                nc.vector.tensor_reduce(
                    rsum[:, :nh, None], e_h[:, :nh, :], mybir.AxisListType.X, ALU.add
                )
                rinv = sp.tile([128, NHM], dt.float32, tag="rinv")
                nc.vector.reciprocal(rinv[:, :nh], rsum[:, :nh])
                wt = sp.tile([128, NHM, NJ], dt.float32, tag="wt")
                nc.vector.scalar_tensor_tensor(
                    wt[:, :nh, :],
                    rinv[:, :nh, None].broadcast_to([128, nh, NJ]), 1.0,
                    e_h[:, :nh, :], ALU.bypass, ALU.mult,
                )
                nc.vector.tensor_mul(wt[:, :nh, :], wt[:, :nh, :], gauss[:, :nh, :])
                wt16 = sp.tile([128, NHM, NJ], dt.float16, tag="wt16")
                nc.vector.tensor_copy(wt16[:, :nh, :], wt[:, :nh, :])
                dall = dp.tile([128, NHM, NJ, 128], dt.float16, tag="dall")
                nc.vector.tensor_tensor(
                    dall[:, :nh, :, :],
                    id128ht[:, None, None, :].broadcast_to([128, nh, NJ, 128]),
                    wt16[:, :nh, :, None].broadcast_to([128, nh, NJ, 128]),
                    ALU.mult,
                )
                st[s]["dall"] = dall

            def phase_F(s):
                """weighted sum via diagonal fp16 matmuls + store."""
                b, ms = STAGES[s]
                gts, dall = st[s]["gts"], st[s]["dall"]
                for i, m in enumerate(ms):
                    wps = wsp.tile([128, QS], dt.float32, tag="wps", space="PSUM")
                    for j in range(NJ):
                        nc.tensor.matmul(
                            wps[:], dall[:, i, j, :], gts[i][:, j, :],
                            start=(j == 0), stop=(j == NJ - 1),
                        )
                    outt = op.tile([128, QS], dt.float16, tag="outt")
                    nc.scalar.activation(outt[:], wps[:], AF.Copy)
                    nc.sync.dma_start(out[b, m * 128 : (m + 1) * 128, :], outt[:])

            # ---- staggered emission: overlap stages across engine queues ----
            sched = []
            for s in range(NSTG):
                sched += [(phase_A, s), (phase_B, s), (phase_C, s)]
                if s >= 1:
                    sched += [(phase_D, s - 1), (phase_E, s - 1)]
                if s >= 2:
                    sched += [(phase_F, s - 2)]
            sched += [(phase_D, NSTG - 1), (phase_E, NSTG - 1)]
            sched += [(phase_F, NSTG - 2), (phase_F, NSTG - 1)]
            for fn, s in sched:
                fn(s)

    nc.compile()
    return nc


def _host_prep(q, c_t, W_a, W_p, V_p):
    q = np.asarray(q, dtype=np.float32)
    c_t = np.asarray(c_t, dtype=np.float32)
    W_a = np.asarray(W_a, dtype=np.float32)
    W_p = np.asarray(W_p, dtype=np.float32)
    V_p = np.asarray(V_p, dtype=np.float32)

    qT16 = np.ascontiguousarray(q.transpose(0, 2, 1)).astype(np.float16)
    cT = np.ascontiguousarray(c_t.transpose(0, 2, 1))
    cT1 = cT.astype(np.float16)
    cT2 = (cT - cT1.astype(np.float32)).astype(np.float16)
    wpT = np.ascontiguousarray(W_p.T)
    wp1 = wpT.astype(np.float16)
    wp2 = (wpT - wp1.astype(np.float32)).astype(np.float16)
    wa1 = W_a.astype(np.float16)
    vpr = np.ascontiguousarray(np.tile(V_p.reshape(1, PS), (128, 1)), dtype=np.float32)
    offs = np.tile(np.arange(-3, 4, dtype=np.float32).reshape(1, 1, NJ), (128, NT, 1))
    offs = np.ascontiguousarray(offs.reshape(128, NT * NJ))
    modm = np.zeros((128, 128), dtype=np.float32)
    for pp in range(128):
        for p in range(128):
            if pp % 16 == p % 16:
                modm[pp, p] = 1.0
    blk16 = np.zeros((128, 8), dtype=np.float32)
    for pp in range(128):
        blk16[pp, pp // 16] = 1.0
    id128h = np.eye(128).astype(np.float16)

    consts = dict(wp1=wp1, wp2=wp2, wa1=wa1, vpr=vpr, offs=offs, modm=modm,
                  blk16=blk16, id128h=id128h)
    in_maps = []
    for k in range(NCORE):
        sl = slice(k * BPC, (k + 1) * BPC)
        m = dict(consts)
        m["qT16"] = np.ascontiguousarray(qT16[sl])
        m["cT1"] = np.ascontiguousarray(cT1[sl])
        m["cT2"] = np.ascontiguousarray(cT2[sl])
        in_maps.append(m)
    return in_maps


def kernel(q, c_t, W_a, W_p, V_p):
    global LAST_EXEC_NS, LAST_RES
    if "nc" not in _CACHE:
        _CACHE["nc"] = _build_nc()
    nc = _CACHE["nc"]
    in_maps = _host_prep(q, c_t, W_a, W_p, V_p)
    res = run_bass_kernel_spmd(nc, in_maps, core_ids=list(range(NCORE)))
    LAST_RES = res
    LAST_EXEC_NS = res.exec_time_ns
    outs = [res.results[k]["out"] for k in range(NCORE)]
    return np.concatenate(outs, axis=0).astype(np.float32)


# revision 22
# speedup vs baseline: 1.0524x; 1.0278x over previous
"""LocalAttention1d Trainium2 kernel.

Layout strategy (B=16 sharded over 8 cores, 2 batches/core):
  - p_t chain in ~fp32 precision: h = tanh(c@W_p.T) via fp16x2 split matmuls
    (c = c1+c2 fp16 pair, W likewise; 3 cross terms give ~1e-7 rel accuracy),
    logit = <tanh(h), V_p> via fused DVE multiply-reduce in fp32.
  - windowed gather: p_int -> int16 block-start indices -> SWDGE dma_gather
    of 7 contiguous q^T rows (one 7KB descriptor per t) from DRAM fp16.
  - scores: fused DVE multiply-reduce (fp16) against u = c@W_a.
  - softmax*gauss -> 7 diagonal fp16 matmuls accumulate the weighted sum in
    PSUM (t-partition layout).
  - software pipeline over 6 variable-size stages (tile groups 2/3/3|3/3/2)
    with staggered emission so PE/DVE/GPSIMD queues overlap across stages.
"""

import sys

sys.path.insert(0, "/opt/trn_rl_repo")

import numpy as np

import bass_rust
import concourse.bass as bass
import concourse.tile as tile
from concourse import bacc, mybir
from concourse.bass_utils import run_bass_kernel_spmd

B, T, S, QS, CS, PS, D = 16, 1024, 4096, 512, 512, 512, 3
NCORE = 8
BPC = B // NCORE  # batches per core
NJ = 2 * D + 1  # 7 window positions
NT = T // 128  # 8 t-tiles per batch
STAGES = [(0, [0, 1]), (0, [2, 3]), (0, [4, 5, 6, 7]),
          (1, [0, 1, 2, 3]), (1, [4, 5]), (1, [6, 7])]
NSTG = len(STAGES)
NHM = 4  # max tiles per stage

dt = mybir.dt
AF = mybir.ActivationFunctionType
ALU = mybir.AluOpType

LAST_EXEC_NS = None
LAST_RES = None
_CACHE = {}


def _build_nc():
    nc = bacc.Bacc("TRN2", target_bir_lowering=False, debug=False, num_devices=NCORE)

    qT16 = nc.dram_tensor("qT16", [BPC, S, QS], dt.float16, kind="ExternalInput").ap()
    cT1 = nc.dram_tensor("cT1", [BPC, CS, T], dt.float16, kind="ExternalInput").ap()
    cT2 = nc.dram_tensor("cT2", [BPC, CS, T], dt.float16, kind="ExternalInput").ap()
    wp1 = nc.dram_tensor("wp1", [CS, PS], dt.float16, kind="ExternalInput").ap()
    wp2 = nc.dram_tensor("wp2", [CS, PS], dt.float16, kind="ExternalInput").ap()
    wa1 = nc.dram_tensor("wa1", [CS, QS], dt.float16, kind="ExternalInput").ap()
    vpr = nc.dram_tensor("vpr", [128, PS], dt.float32, kind="ExternalInput").ap()
    offs = nc.dram_tensor("offs", [128, NT * NJ], dt.float32, kind="ExternalInput").ap()
    modm = nc.dram_tensor("modm", [128, 128], dt.float32, kind="ExternalInput").ap()
    blk16 = nc.dram_tensor("blk16", [128, 8], dt.float32, kind="ExternalInput").ap()
    id128h = nc.dram_tensor("id128h", [128, 128], dt.float16, kind="ExternalInput").ap()
    out = nc.dram_tensor("out", [BPC, T, QS], dt.float16, kind="ExternalOutput").ap()

    with tile.TileContext(nc) as tc:
        import contextlib

        ctx = contextlib.ExitStack()
        with ctx:
            cpool = ctx.enter_context(tc.tile_pool(name="consts", bufs=1))
            ctp = ctx.enter_context(tc.tile_pool(name="ct", bufs=16))
            gp = ctx.enter_context(tc.tile_pool(name="gath", bufs=11))
            up = ctx.enter_context(tc.tile_pool(name="u16", bufs=9))
            sp = ctx.enter_context(tc.tile_pool(name="small", bufs=3))
            dp = ctx.enter_context(tc.tile_pool(name="dall", bufs=2))
            gtp = ctx.enter_context(tc.tile_pool(name="tanh", bufs=2))
            jp = ctx.enter_context(tc.tile_pool(name="junk", bufs=4))
            op = ctx.enter_context(tc.tile_pool(name="outp", bufs=2))
            mmp = ctx.enter_context(tc.tile_pool(name="mm", bufs=2, space="PSUM"))
            wsp = ctx.enter_context(tc.tile_pool(name="ws", bufs=2, space="PSUM"))
            tpp = ctx.enter_context(tc.tile_pool(name="tp", bufs=2, space="PSUM"))

            def _floor_into(dst, src, sfx, shape):
                """Exact floor(src) for src >= 0 written into dst; src/dst are
                equally-shaped APs, scratch allocated at `shape` and sliced."""
                sl = (slice(None),) + tuple(slice(0, n) for n in src.shape[1:])
                i32 = sp.tile(shape, dt.int32, tag="fli" + sfx)
                iv = i32[sl]
                nc.vector.tensor_copy(iv, src)
                cand = sp.tile(shape, dt.float32, tag="flc" + sfx)
                cv = cand[sl]
                nc.vector.tensor_copy(cv, iv)
                corr = sp.tile(shape, dt.float32, tag="flx" + sfx)
                rv = corr[sl]
                nc.vector.scalar_tensor_tensor(rv, cv, 1.0, src, ALU.bypass, ALU.is_gt)
                nc.vector.tensor_tensor(dst, cv, rv, ALU.subtract)

            # ---- constants + c loads on sync queue, dependency order ----
            wp1t = cpool.tile([128, 4, PS], dt.float16)
            wp2t = cpool.tile([128, 4, PS], dt.float16)
            for k in range(4):
                nc.sync.dma_start(wp1t[:, k, :], wp1[k * 128 : (k + 1) * 128, :])
                nc.sync.dma_start(wp2t[:, k, :], wp2[k * 128 : (k + 1) * 128, :])

            all_ct1s, all_ct2s = [[], []], [[], []]
            for k in range(4):
                c1t = ctp.tile([128, T], dt.float16, tag="ct1")
                nc.sync.dma_start(c1t[:], cT1[0, k * 128 : (k + 1) * 128, :])
                all_ct1s[0].append(c1t)
            for k in range(4):
                c2t = ctp.tile([128, T], dt.float16, tag="ct2")
                nc.sync.dma_start(c2t[:], cT2[0, k * 128 : (k + 1) * 128, :])
                all_ct2s[0].append(c2t)
            vprt = cpool.tile([128, PS], dt.float32)
            nc.sync.dma_start(vprt[:], vpr[:])
            modmt = cpool.tile([128, 128], dt.float32)
            nc.sync.dma_start(modmt[:], modm[:])
            blk16t = cpool.tile([128, 8], dt.float32)
            nc.sync.dma_start(blk16t[:], blk16[:])
            wa1t = cpool.tile([128, 4, QS], dt.float16)
            nc.sync.dma_start(wa1t[:], wa1[:].rearrange("(k p) n -> p k n", p=128))
            offst = cpool.tile([128, NT * NJ], dt.float32)
            nc.sync.dma_start(offst[:], offs[:])
            id128ht = cpool.tile([128, 128], dt.float16)
            nc.sync.dma_start(id128ht[:], id128h[:])
            for k in range(4):
                c1t = ctp.tile([128, T], dt.float16, tag="ct1")
                nc.sync.dma_start(c1t[:], cT1[1, k * 128 : (k + 1) * 128, :])
                all_ct1s[1].append(c1t)
            for k in range(4):
                c2t = ctp.tile([128, T], dt.float16, tag="ct2")
                nc.sync.dma_start(c2t[:], cT2[1, k * 128 : (k + 1) * 128, :])
                all_ct2s[1].append(c2t)

            def chunk(t, k):
                return t[:, k, :]

            qwins = []
            for b in range(BPC):
                qw = qT16[b].copy()
                qw.ap = bass_rust.VecI64Pair([[QS, S - NJ + 1], [1, NJ * QS]])
                qwins.append(qw)

            st = [dict() for _ in range(NSTG)]

            def phase_A(s):
                """h matmuls + tanh + logit + perm/idx chain for one stage."""
                b, ms = STAGES[s]
                nh = len(ms)
                ct1s, ct2s = all_ct1s[b], all_ct2s[b]
                lg = sp.tile([128, NHM], dt.float32, tag="lg")
                st[s]["lg"] = lg
                for i, m in enumerate(ms):
                    hps = mmp.tile([128, PS], dt.float32, tag="hps", space="PSUM")
                    nmm = 0
                    terms = [
                        (ct1s[k][:, m * 128 : (m + 1) * 128], chunk(w, k))
                        for k in range(4)
                        for w in (wp1t, wp2t)
                    ] + [
                        (ct2s[k][:, m * 128 : (m + 1) * 128], chunk(wp1t, k))
                        for k in range(4)
                    ]
                    for lhs, rhs in terms:
                        nc.tensor.matmul(hps[:], lhs, rhs, start=(nmm == 0), stop=(nmm == 11))
                        nmm += 1
                    g = gtp.tile([128, PS], dt.float32, tag="g")
                    nc.scalar.activation(g[:], hps[:], AF.Tanh)
                    junkf = jp.tile([128, PS], dt.float16, tag="junkf")
                    nc.vector.scalar_tensor_tensor(
                        junkf[:], g[:], 1.0, vprt[:], ALU.bypass, ALU.mult,
                        accum_out=lg[:, i : i + 1],
                    )
                # t-layout p_t chain once: sigmoid -> x4096 -> exact floor
                sig8 = sp.tile([128, NHM], dt.float32, tag="sig8")
                nc.scalar.activation(sig8[:, :nh], lg[:, :nh], AF.Sigmoid)
                i32 = sp.tile([128, NHM], dt.int32, tag="fli8")
                nc.vector.tensor_scalar_mul(i32[:, :nh], sig8[:, :nh], 4096.0)
                cand = sp.tile([128, NHM], dt.float32, tag="flc8")
                nc.vector.tensor_copy(cand[:, :nh], i32[:, :nh])
                corr = sp.tile([128, NHM], dt.float32, tag="flx8")
                nc.vector.scalar_tensor_tensor(
                    corr[:, :nh], cand[:, :nh], 1.0 / 4096.0, sig8[:, :nh],
                    ALU.mult, ALU.is_gt,
                )
                pi8 = sp.tile([128, NHM], dt.float32, tag="pi8")
                nc.vector.tensor_tensor(pi8[:, :nh], cand[:, :nh], corr[:, :nh], ALU.subtract)
                pt8 = sp.tile([128, NHM], dt.float32, tag="pt8")
                nc.vector.tensor_scalar_mul(pt8[:, :nh], sig8[:, :nh], 4096.0)
                st[s]["pi8"] = pi8
                st[s]["pt8"] = pt8
                # wrapped-16 permute of the floored p_int via one mask-matmul:
                # pw[p, (i,w)] = sum_p' 1[p' % 16 == p % 16] (pi8[p',i] blk[p',w])
                prh = sp.tile([128, NHM, 8], dt.float32, tag="prh")
                nc.vector.tensor_tensor(
                    prh[:, :nh, :],
                    pi8[:, :nh, None].broadcast_to([128, nh, 8]),
                    blk16t[:, None, :].broadcast_to([128, nh, 8]),
                    ALU.mult,
                )
                pps = tpp.tile([128, NHM, 8], dt.float32, tag="pps", space="PSUM")
                nc.tensor.matmul(
                    pps[:, :nh, :], modmt[:], prh[:, :nh, :], start=True, stop=True
                )
                idxs = sp.tile([128, NHM, 8], dt.int16, tag="idxs")
                tmpp = sp.tile([128, NHM, 8], dt.float32, tag="tmpp")
                nc.vector.tensor_scalar(
                    tmpp[:, :nh, :], pps[:, :nh, :], 3.0, 4092.0, ALU.max, ALU.min
                )
                nc.vector.tensor_scalar(
                    idxs[:, :nh, :], tmpp[:, :nh, :], -3.0, None, ALU.add
                )
                st[s]["idxs"] = idxs

            def phase_B(s):
                """gathers (gpsimd queue) + t-layout gauss/mask prep."""
                b, ms = STAGES[s]
                nh = len(ms)
                idxs = st[s]["idxs"]
                gts = []
                for i, m in enumerate(ms):
                    gt = gp.tile([128, 1, NJ * QS], dt.float16, tag="gt")
                    nc.gpsimd.dma_gather(
                        gt[:], qwins[b], idxs[:, i, :], 128, 128, NJ * QS,
                        elem_step=QS, single_packet=False,
                    )
                    gts.append(gt[:, 0, :].rearrange("p (j q) -> p j q", j=NJ))
                st[s]["gts"] = gts

                pi8, pt8 = st[s]["pi8"], st[s]["pt8"]

                pos_all = sp.tile([128, NHM, NJ], dt.float32, tag="pos_all")
                pos3 = pos_all[:, :nh, :]
                nc.vector.scalar_tensor_tensor(
                    pos3, pi8[:, :nh, None].broadcast_to([128, nh, NJ]), 1.0,
                    offst[:].rearrange("p (m j) -> p m j", j=NJ)[:, :nh, :],
                    ALU.bypass, ALU.add,
                )
                dtile = sp.tile([128, NHM, NJ], dt.float32, tag="dtile")
                nc.vector.scalar_tensor_tensor(
                    dtile[:, :nh, :],
                    pt8[:, :nh, None].broadcast_to([128, nh, NJ]), 1.0,
                    pos3, ALU.bypass, ALU.subtract,
                )
                # gauss = exp(-(2/9) d^2); square on DVE keeps ACT table set small
                g1 = sp.tile([128, NHM, NJ], dt.float32, tag="g1")
                nc.vector.tensor_tensor(
                    g1[:, :nh, :], dtile[:, :nh, :], dtile[:, :nh, :], ALU.mult
                )
                st[s]["g1"] = g1
                m1 = sp.tile([128, NHM, NJ], dt.float32, tag="m1")
                nc.vector.tensor_scalar(
                    m1[:, :nh, :], pos_all[:, :nh, :], 0.0, -1e30, ALU.is_lt, ALU.mult
                )
                maskb = sp.tile([128, NHM, NJ], dt.float32, tag="maskb")
                nc.vector.tensor_scalar(
                    maskb[:, :nh, :], pos_all[:, :nh, :], 4095.0, -1e30, ALU.is_gt, ALU.mult
                )
                nc.vector.tensor_add(maskb[:, :nh, :], maskb[:, :nh, :], m1[:, :nh, :])
                st[s]["maskb"] = maskb

            def phase_C(s):
                """u = c1 @ W_a for the stage (PE, overlaps gather DMA)."""
                b, ms = STAGES[s]
                ct1s = all_ct1s[b]
                u16s = []
                for m in ms:
                    ups = mmp.tile([128, QS], dt.float32, tag="ups", space="PSUM")
                    for k in range(4):
                        nc.tensor.matmul(
                            ups[:], ct1s[k][:, m * 128 : (m + 1) * 128], chunk(wa1t, k),
                            start=(k == 0), stop=(k == 3),
                        )
                    u16 = up.tile([128, QS], dt.float16, tag="u16")
                    nc.scalar.activation(u16[:], ups[:], AF.Copy)
                    u16s.append(u16)
                st[s]["u16s"] = u16s

            def phase_D(s):
                """scores: fused multiply-reduce per (tile, j)."""
                nh = len(STAGES[s][1])
                gts, u16s = st[s]["gts"], st[s]["u16s"]
                a_h = sp.tile([128, NHM, NJ], dt.float32, tag="a_h")
                for i in range(nh):
                    for j in range(NJ):
                        col = a_h[:, i, j : j + 1]
                        if (i * NJ + j) % 7 < 4:
                            junk16 = jp.tile([128, QS], dt.float16, tag="junk16")
                            nc.vector.scalar_tensor_tensor(
                                junk16[:], gts[i][:, j, :], 1.0, u16s[i][:],
                                ALU.bypass, ALU.mult, accum_out=col,
                            )
                        else:
                            prod = jp.tile([128, QS], dt.float16, tag="prod")
                            nc.vector.tensor_tensor(
                                prod[:], gts[i][:, j, :], u16s[i][:], ALU.mult
                            )
                            junka = jp.tile([128, QS], dt.float16, tag="junka")
                            nc.scalar.activation(
                                junka[:], prod[:], AF.Copy, accum_out=col
                            )
                st[s]["a_h"] = a_h

            def phase_E(s):
                """masked softmax * gauss -> fp16 diag weights."""
                nh = len(STAGES[s][1])
                a_h, maskb = st[s]["a_h"], st[s]["maskb"]
                a3 = a_h[:, :nh, :]
                nc.vector.tensor_add(a3, a3, maskb[:, :nh, :])
                rmax = sp.tile([128, NHM], dt.float32, tag="rmax")
                nc.vector.tensor_reduce(rmax[:, :nh, None], a3, mybir.AxisListType.X, ALU.max)
                asub = sp.tile([128, NHM, NJ], dt.float32, tag="asub")
                nc.vector.scalar_tensor_tensor(
                    asub[:, :nh, :],
                    rmax[:, :nh, None].broadcast_to([128, nh, NJ]), 1.0,
                    a3, ALU.bypass, ALU.subtract,
                )
                e_h = sp.tile([128, NHM, NJ], dt.float32, tag="e_h")
                nc.scalar.activation(e_h[:, :nh, :], asub[:, :nh, :], AF.Exp, scale=-1.0)
                gauss = sp.tile([128, NHM, NJ], dt.float32, tag="gauss")
                nc.scalar.activation(
                    gauss[:, :nh, :], st[s]["g1"][:, :nh, :], AF.Exp, scale=-2.0 / 9.0
                )
                rsum = sp.tile([128, NHM], dt.float32, tag="rsum")
                nc.vector.tensor_reduce(
                    rsum[:, :nh, None], e_h[:, :nh, :], mybir.AxisListType.X, ALU.add
                )
                rinv = sp.tile([128, NHM], dt.float32, tag="rinv")
                nc.vector.reciprocal(rinv[:, :nh], rsum[:, :nh])
                wt = sp.tile([128, NHM, NJ], dt.float32, tag="wt")
                nc.vector.scalar_tensor_tensor(
                    wt[:, :nh, :],
                    rinv[:, :nh, None].broadcast_to([128, nh, NJ]), 1.0,
                    e_h[:, :nh, :], ALU.bypass, ALU.mult,
                )
                nc.vector.tensor_mul(wt[:, :nh, :], wt[:, :nh, :], gauss[:, :nh, :])
                wt16 = sp.tile([128, NHM, NJ], dt.float16, tag="wt16")
                nc.vector.tensor_copy(wt16[:, :nh, :], wt[:, :nh, :])
                dall = dp.tile([128, NHM, NJ, 128], dt.float16, tag="dall")
                nc.vector.tensor_tensor(
                    dall[:, :nh, :, :],
                    id128ht[:, None, None, :].broadcast_to([128, nh, NJ, 128]),
                    wt16[:, :nh, :, None].broadcast_to([128, nh, NJ, 128]),
                    ALU.mult,
                )
                st[s]["dall"] = dall

            def phase_F(s):
                """weighted sum via diagonal fp16 matmuls + store."""
                b, ms = STAGES[s]
                gts, dall = st[s]["gts"], st[s]["dall"]
                for i, m in enumerate(ms):
                    wps = wsp.tile([128, QS], dt.float32, tag="wps", space="PSUM")
                    for j in range(NJ):
                        nc.tensor.matmul(
                            wps[:], dall[:, i, j, :], gts[i][:, j, :],
                            start=(j == 0), stop=(j == NJ - 1),
                        )
                    outt = op.tile([128, QS], dt.float16, tag="outt")
                    nc.scalar.activation(outt[:], wps[:], AF.Copy)
                    nc.sync.dma_start(out[b, m * 128 : (m + 1) * 128, :], outt[:])

            # ---- staggered emission: overlap stages across engine queues ----
            sched = []
            for s in range(NSTG):
                sched += [(phase_A, s), (phase_B, s), (phase_C, s)]
                if s >= 1:
                    sched += [(phase_D, s - 1), (phase_E, s - 1)]
                if s >= 2:
                    sched += [(phase_F, s - 2)]
            sched += [(phase_D, NSTG - 1), (phase_E, NSTG - 1)]
            sched += [(phase_F, NSTG - 2), (phase_F, NSTG - 1)]
            for fn, s in sched:
                fn(s)

    nc.compile()
    return nc


def _host_prep(q, c_t, W_a, W_p, V_p):
    q = np.asarray(q, dtype=np.float32)
    c_t = np.asarray(c_t, dtype=np.float32)
    W_a = np.asarray(W_a, dtype=np.float32)
    W_p = np.asarray(W_p, dtype=np.float32)
    V_p = np.asarray(V_p, dtype=np.float32)

    qT16 = np.ascontiguousarray(q.transpose(0, 2, 1)).astype(np.float16)
    cT = np.ascontiguousarray(c_t.transpose(0, 2, 1))
    cT1 = cT.astype(np.float16)
    cT2 = (cT - cT1.astype(np.float32)).astype(np.float16)
    wpT = np.ascontiguousarray(W_p.T)
    wp1 = wpT.astype(np.float16)
    wp2 = (wpT - wp1.astype(np.float32)).astype(np.float16)
    wa1 = W_a.astype(np.float16)
    vpr = np.ascontiguousarray(np.tile(V_p.reshape(1, PS), (128, 1)), dtype=np.float32)
    offs = np.tile(np.arange(-3, 4, dtype=np.float32).reshape(1, 1, NJ), (128, NT, 1))
    offs = np.ascontiguousarray(offs.reshape(128, NT * NJ))
    modm = np.zeros((128, 128), dtype=np.float32)
    for pp in range(128):
        for p in range(128):
            if pp % 16 == p % 16:
                modm[pp, p] = 1.0
    blk16 = np.zeros((128, 8), dtype=np.float32)
    for pp in range(128):
        blk16[pp, pp // 16] = 1.0
    id128h = np.eye(128).astype(np.float16)

    consts = dict(wp1=wp1, wp2=wp2, wa1=wa1, vpr=vpr, offs=offs, modm=modm,
                  blk16=blk16, id128h=id128h)
    in_maps = []
    for k in range(NCORE):
        sl = slice(k * BPC, (k + 1) * BPC)
        m = dict(consts)
        m["qT16"] = np.ascontiguousarray(qT16[sl])
        m["cT1"] = np.ascontiguousarray(cT1[sl])
        m["cT2"] = np.ascontiguousarray(cT2[sl])
        in_maps.append(m)
    return in_maps


def kernel(q, c_t, W_a, W_p, V_p):
    global LAST_EXEC_NS, LAST_RES
    if "nc" not in _CACHE:
        _CACHE["nc"] = _build_nc()
    nc = _CACHE["nc"]
    in_maps = _host_prep(q, c_t, W_a, W_p, V_p)
    res = run_bass_kernel_spmd(nc, in_maps, core_ids=list(range(NCORE)))
    LAST_RES = res
    LAST_EXEC_NS = res.exec_time_ns
    outs = [res.results[k]["out"] for k in range(NCORE)]
    return np.concatenate(outs, axis=0).astype(np.float32)
